# revision 6
# baseline (speedup 1.0000x reference)
"""Trainium2 Bass kernel for nn_NodeCriticalityGNN_4595615006784.

Mathematical derivation (why this kernel is exact, for ALL inputs)
------------------------------------------------------------------
The reference network ends in five "ResidualMLP" heads:

    def _resmlp(x, f1w, f1b, f2w, f2b, nw, nb, pw, pb):
        hh = _gelu(x @ f1w + f1b)
        hh = hh @ f2w + f2b
        return _layernorm(hh + x @ pw + pb, nw, nb)

    rmav[i] = sigmoid(_resmlp(h, ...))        # fc2 maps C//2 -> 1
    comp    = sigmoid(_resmlp(comp_in, ...))  # fc2 maps C//2 -> 1

Every head's _resmlp output has feature dimension 1 (hfc2_w: [C//2, 1],
cfc2_w: [C//2, 1], hproj_w/cproj_w: [*, 1]).  _layernorm normalizes over
the LAST axis:

    mu  = mean(x, axis=-1)          # over a SINGLE element -> mu == x
    var = mean((x - mu)**2) == 0    # exactly, in floating point
    out = (x - mu) / sqrt(var + 1e-5) * w + b
        = 0 / sqrt(1e-5) * w + b
        = b                          # exactly (0*w == 0, 0 + b == b)

`mean` over one element divides by 1 (no rounding), so (x - mu) is an
exact floating-point zero for every input.  Hence each head output is
exactly its LayerNorm bias, independent of h, x, edges, and every other
weight.  Therefore, for ALL possible inputs:

    out[n, 0]     = sigmoid(cnorm_b[0])
    out[n, 1 + i] = sigmoid(hnorm_b[i, 0])    for i in 0..3, for every n

The entire GAT message-passing stack is dead code — its output is
multiplied by an exact zero.  This was verified numerically against
reference.py: perturbing x / edge_attr / any GNN weight changes the
output by exactly 0.0, while perturbing hnorm_b / cnorm_b changes it
exactly as sigmoid(bias) predicts (hnorm_w has no effect, as derived).

The optimal memory-regime kernel therefore reads the 5 bias scalars,
applies sigmoid on-device (ScalarE LUT), and broadcasts to the [N, 5]
output.  Work is sharded row-wise across the 8 NeuronCores: core k
produces output rows [k*12500, (k+1)*12500).

Device kernel per core (3 instructions + sync):
  1. DMA:   head_bias [128, 5] f32  HBM -> SBUF
  2. ACT:   sigmoid with a stride-0 broadcast access pattern,
            [128, 5] -> [128, 98, 5]  (12544 rows worth per core)
  3. DMA:   SBUF [128, 490] -> HBM out [128, 490]
Host reshapes [128, 490] -> [12544, 5], takes the first 12500 rows per
core and concatenates the 8 shards -> [100000, 5].
"""

import os
import sys

import numpy as np

# Hardcoded problem shape (kernel.py must be self-contained).
N = 100000
N_CORES = 8
ROWS_PER_CORE = N // N_CORES          # 12500
PART = 128                            # SBUF partitions
GROUPS = 98                           # 128 * 98 = 12544 >= 12500
ROWS_PAD = PART * GROUPS              # 12544

for _p in ("/opt/trn_rl_repo", "/root/.axon_site/_ro/trn_rl_repo"):
    if os.path.isdir(_p) and _p not in sys.path:
        sys.path.append(_p)

from concourse import bass, mybir  # noqa: E402
from concourse.bass import AP  # noqa: E402
from concourse.bass_utils import run_bass_kernel_spmd  # noqa: E402

# Stash of the last run's BassKernelResults (exec_time_ns etc.) so a
# harness/test can read profiling info without changing kernel()'s API.
LAST_RESULT = None


def _build_bass():
    """Per-core program: out[p, g, :] = sigmoid(head_bias[p, :])."""
    nc = bass.Bass()
    bias_in = nc.declare_dram_parameter(
        "head_bias", [PART, 5], mybir.dt.float32, isOutput=False
    )
    out_ext = nc.declare_dram_parameter(
        "out", [PART, GROUPS * 5], mybir.dt.float32, isOutput=True
    )

    with (
        nc.sbuf_tensor("sb_bias", [PART, 5], mybir.dt.float32) as sb_bias,
        nc.sbuf_tensor("sb_out", [PART, GROUPS * 5], mybir.dt.float32) as sb_out,
        nc.sbuf_tensor("sb_scratch", [PART, 1], mybir.dt.float32) as sb_scratch,
        nc.Block() as block,
        nc.semaphore("dma_sem") as dma_sem,
    ):
        # Single-engine program on ScalarE (which can issue HWDGE DMAs):
        # no cross-engine semaphore hops at all.
        @block.scalar
        def _(scalar):
            # 1. Input DMA first, so its ~2 us completion latency overlaps
            #    with the sigmoid ACT-table load triggered by the dummy
            #    activation below (preload_activation_table trick).
            scalar.dma_start(out=sb_bias[:], in_=bias_in[:]).then_inc(dma_sem, 16)
            scalar.memzero(sb_scratch[:])
            scalar.drain()
            scalar.activation(
                out=sb_scratch[:],
                in_=sb_scratch[:],
                func=mybir.ActivationFunctionType.Sigmoid,
            )
            # 2. Real compute: read [128, 5] with a stride-0 middle axis,
            #    write sigmoid into [128, 98, 5] (12544 output rows).
            scalar.wait_ge(dma_sem, 16)
            in_bcast = AP(sb_bias[:].tensor, 0, [[5, PART], [0, GROUPS], [1, 5]])
            out_view = AP(
                sb_out[:].tensor, 0, [[GROUPS * 5, PART], [5, GROUPS], [1, 5]]
            )
            scalar.activation(
                out=out_view,
                in_=in_bcast,
                func=mybir.ActivationFunctionType.Sigmoid,
            )
            # 3. Output DMA; drain first so the ACT write to sb_out is
            #    retired before the DMA engines read it, then wait for the
            #    write receipt before retiring.
            scalar.drain()
            scalar.dma_start(out=out_ext[:], in_=sb_out[:]).then_inc(dma_sem, 16)
            scalar.wait_ge(dma_sem, 32)

    return nc


def kernel(**inputs) -> np.ndarray:
    global LAST_RESULT

    hnorm_b = np.asarray(inputs["hnorm_b"], dtype=np.float32).reshape(4)
    cnorm_b = np.asarray(inputs["cnorm_b"], dtype=np.float32).reshape(1)
    bias_row = np.concatenate([cnorm_b, hnorm_b])  # [5]: comp, rmav0..3
    head_bias = np.ascontiguousarray(
        np.broadcast_to(bias_row[None, :], (PART, 5)), dtype=np.float32
    )

    nc = _build_bass()
    # Shard rows across the 8 cores; the row->value map is constant in n,
    # so every core receives the same (replicated) bias tile and computes
    # its 12544-row slab; the host keeps 12500 rows per core.
    in_maps = [{"head_bias": head_bias} for _ in range(N_CORES)]
    trace = os.environ.get("KERNEL_TRACE", "0") == "1"
    res = run_bass_kernel_spmd(
        nc, in_maps, core_ids=list(range(N_CORES)), trace=trace
    )
    LAST_RESULT = res

    shards = []
    for k in range(N_CORES):
        tile = np.asarray(res.results[k]["out"], dtype=np.float32)
        shards.append(tile.reshape(ROWS_PAD, 5)[:ROWS_PER_CORE])
    return np.ascontiguousarray(np.concatenate(shards, axis=0))


if __name__ == "__main__":
    rng = np.random.default_rng(0)
    demo = {
        "hnorm_b": np.zeros((4, 1), np.float32),
        "cnorm_b": np.zeros((1,), np.float32),
    }
    out = kernel(**demo)
    print("out", out.shape, out.dtype, "max|out-0.5| =", np.abs(out - 0.5).max())


# revision 9
# speedup vs baseline: 1.1321x; 1.1321x over previous
"""Trainium2 Bass kernel for nn_NodeCriticalityGNN_4595615006784.

Mathematical derivation (why this kernel is exact, for ALL inputs)
------------------------------------------------------------------
The reference network ends in five "ResidualMLP" heads:

    def _resmlp(x, f1w, f1b, f2w, f2b, nw, nb, pw, pb):
        hh = _gelu(x @ f1w + f1b)
        hh = hh @ f2w + f2b
        return _layernorm(hh + x @ pw + pb, nw, nb)

    rmav[i] = sigmoid(_resmlp(h, ...))        # fc2 maps C//2 -> 1
    comp    = sigmoid(_resmlp(comp_in, ...))  # fc2 maps C//2 -> 1

Every head's _resmlp output has feature dimension 1 (hfc2_w: [C//2, 1],
cfc2_w: [C//2, 1], hproj_w/cproj_w: [*, 1]).  _layernorm normalizes over
the LAST axis:

    mu  = mean(x, axis=-1)          # over a SINGLE element -> mu == x
    var = mean((x - mu)**2) == 0    # exactly, in floating point
    out = (x - mu) / sqrt(var + 1e-5) * w + b
        = 0 / sqrt(1e-5) * w + b
        = b                          # exactly (0*w == 0, 0 + b == b)

`mean` over one element divides by 1 (no rounding), so (x - mu) is an
exact floating-point zero for every input.  Hence each head output is
exactly its LayerNorm bias, independent of h, x, edges, and every other
weight.  Therefore, for ALL possible inputs:

    out[n, 0]     = sigmoid(cnorm_b[0])
    out[n, 1 + i] = sigmoid(hnorm_b[i, 0])    for i in 0..3, for every n

The entire GAT message-passing stack is dead code — its output is
multiplied by an exact zero.  This was verified numerically against
reference.py: perturbing x / edge_attr / any GNN weight changes the
output by exactly 0.0, while perturbing hnorm_b / cnorm_b changes it
exactly as sigmoid(bias) predicts (hnorm_w has no effect, as derived).

The optimal memory-regime kernel therefore reads the 5 bias scalars,
applies sigmoid on-device (ScalarE LUT), and broadcasts to the [N, 5]
output.  Work is sharded row-wise across the 8 NeuronCores: core k
produces output rows [k*12500, (k+1)*12500).

Device kernel per core (single-engine program on ScalarE, which can
issue HWDGE DMAs — no cross-engine semaphore hops):
  1. DMA:   head_bias [128, 5] f32  HBM -> SBUF  (completion ~2 us)
  2. dummy sigmoid on scratch: hoists the sigmoid ACT-table load into
     the DMA-completion window instead of the critical path
  3. ACT:   sigmoid with a stride-0 broadcast access pattern,
            [128, 5] -> [128, 98, 5]  (12544 rows worth per core)
  4. DMA:   SBUF [128, 490] -> HBM out [128, 490], wait for receipt
Host reshapes [128, 490] -> [12544, 5], takes the first 12500 rows per
core and concatenates the 8 shards -> [100000, 5].

Measured (neuron-profile, core 0): ~14.6 us whole-NEFF exec, of which
~9.5 us is the fixed bass prologue/epilogue (engine preambles, semaphore
init, const pool, barriers) and ~5 us is the two serial DMA completion
latencies + ACT work.  Output matches the reference bit-exactly on the
real inputs and to ~1e-6 under perturbed head biases.
"""

import os
import sys

import numpy as np

# Hardcoded problem shape (kernel.py must be self-contained).
N = 100000
N_CORES = 8
ROWS_PER_CORE = N // N_CORES          # 12500
PART = 128                            # SBUF partitions
GROUPS = 98                           # 128 * 98 = 12544 >= 12500
ROWS_PAD = PART * GROUPS              # 12544

for _p in ("/opt/trn_rl_repo", "/root/.axon_site/_ro/trn_rl_repo"):
    if os.path.isdir(_p) and _p not in sys.path:
        sys.path.append(_p)

from concourse import bass, mybir  # noqa: E402
from concourse.bass import AP  # noqa: E402
from concourse.bass_utils import run_bass_kernel_spmd  # noqa: E402

# Stash of the last run's BassKernelResults (exec_time_ns etc.) so a
# harness/test can read profiling info without changing kernel()'s API.
LAST_RESULT = None


def _build_bass():
    """Per-core program: out[p, g, :] = sigmoid(head_bias[p, :])."""
    nc = bass.Bass()
    bias_in = nc.declare_dram_parameter(
        "head_bias", [PART, 5], mybir.dt.float32, isOutput=False
    )
    out_ext = nc.declare_dram_parameter(
        "out", [PART, GROUPS * 5], mybir.dt.float32, isOutput=True
    )

    with (
        nc.sbuf_tensor("sb_bias", [PART, 5], mybir.dt.float32) as sb_bias,
        nc.sbuf_tensor("sb_out", [PART, GROUPS * 5], mybir.dt.float32) as sb_out,
        nc.sbuf_tensor("sb_scratch", [PART, 1], mybir.dt.float32) as sb_scratch,
        nc.Block(no_gpsimd_drain=True) as block,
        nc.semaphore("dma_sem") as dma_sem,
    ):
        # Single-engine program on ScalarE (which can issue HWDGE DMAs):
        # no cross-engine semaphore hops at all.
        @block.scalar
        def _(scalar):
            # 1. Input DMA first, so its ~2 us completion latency overlaps
            #    with the sigmoid ACT-table load triggered by the dummy
            #    activation below (preload_activation_table trick).
            scalar.dma_start(out=sb_bias[:], in_=bias_in[:]).then_inc(dma_sem, 16)
            scalar.memzero(sb_scratch[:])
            scalar.drain()
            scalar.activation(
                out=sb_scratch[:],
                in_=sb_scratch[:],
                func=mybir.ActivationFunctionType.Sigmoid,
            )
            # 2. Real compute: read [128, 5] with a stride-0 middle axis,
            #    write sigmoid into [128, 98, 5] (12544 output rows).
            scalar.wait_ge(dma_sem, 16)
            in_bcast = AP(sb_bias[:].tensor, 0, [[5, PART], [0, GROUPS], [1, 5]])
            out_view = AP(
                sb_out[:].tensor, 0, [[GROUPS * 5, PART], [5, GROUPS], [1, 5]]
            )
            scalar.activation(
                out=out_view,
                in_=in_bcast,
                func=mybir.ActivationFunctionType.Sigmoid,
            )
            # 3. Output DMA; drain first so the ACT write to sb_out is
            #    retired before the DMA engines read it, then wait for the
            #    write receipt before retiring.
            scalar.drain()
            scalar.dma_start(out=out_ext[:], in_=sb_out[:]).then_inc(dma_sem, 16)
            scalar.wait_ge(dma_sem, 32)

    return nc


def kernel(**inputs) -> np.ndarray:
    global LAST_RESULT

    hnorm_b = np.asarray(inputs["hnorm_b"], dtype=np.float32).reshape(4)
    cnorm_b = np.asarray(inputs["cnorm_b"], dtype=np.float32).reshape(1)
    bias_row = np.concatenate([cnorm_b, hnorm_b])  # [5]: comp, rmav0..3
    head_bias = np.ascontiguousarray(
        np.broadcast_to(bias_row[None, :], (PART, 5)), dtype=np.float32
    )

    nc = _build_bass()
    # Shard rows across the 8 cores; the row->value map is constant in n,
    # so every core receives the same (replicated) bias tile and computes
    # its 12544-row slab; the host keeps 12500 rows per core.
    in_maps = [{"head_bias": head_bias} for _ in range(N_CORES)]
    trace = os.environ.get("KERNEL_TRACE", "0") == "1"
    res = run_bass_kernel_spmd(
        nc, in_maps, core_ids=list(range(N_CORES)), trace=trace
    )
    LAST_RESULT = res

    shards = []
    for k in range(N_CORES):
        tile = np.asarray(res.results[k]["out"], dtype=np.float32)
        shards.append(tile.reshape(ROWS_PAD, 5)[:ROWS_PER_CORE])
    return np.ascontiguousarray(np.concatenate(shards, axis=0))


if __name__ == "__main__":
    demo = {
        "hnorm_b": np.zeros((4, 1), np.float32),
        "cnorm_b": np.zeros((1,), np.float32),
    }
    out = kernel(**demo)
    print("out", out.shape, out.dtype, "max|out-0.5| =", np.abs(out - 0.5).max())


# revision 11
# speedup vs baseline: 1.1615x; 1.0260x over previous
"""Trainium2 Bass kernel for nn_NodeCriticalityGNN_4595615006784.

Mathematical derivation (why this kernel is exact, for ALL inputs)
------------------------------------------------------------------
The reference network ends in five "ResidualMLP" heads:

    def _resmlp(x, f1w, f1b, f2w, f2b, nw, nb, pw, pb):
        hh = _gelu(x @ f1w + f1b)
        hh = hh @ f2w + f2b
        return _layernorm(hh + x @ pw + pb, nw, nb)

    rmav[i] = sigmoid(_resmlp(h, ...))        # fc2 maps C//2 -> 1
    comp    = sigmoid(_resmlp(comp_in, ...))  # fc2 maps C//2 -> 1

Every head's _resmlp output has feature dimension 1 (hfc2_w: [C//2, 1],
cfc2_w: [C//2, 1], hproj_w/cproj_w: [*, 1]).  _layernorm normalizes over
the LAST axis:

    mu  = mean(x, axis=-1)          # over a SINGLE element -> mu == x
    var = mean((x - mu)**2) == 0    # exactly, in floating point
    out = (x - mu) / sqrt(var + 1e-5) * w + b
        = 0 / sqrt(1e-5) * w + b
        = b                          # exactly (0*w == 0, 0 + b == b)

`mean` over one element divides by 1 (no rounding), so (x - mu) is an
exact floating-point zero for every input.  Hence each head output is
exactly its LayerNorm bias, independent of h, x, edges, and every other
weight.  Therefore, for ALL possible inputs:

    out[n, 0]     = sigmoid(cnorm_b[0])
    out[n, 1 + i] = sigmoid(hnorm_b[i, 0])    for i in 0..3, for every n

The entire GAT message-passing stack is dead code — its output is
multiplied by an exact zero.  This was verified numerically against
reference.py: perturbing x / edge_attr / any GNN weight changes the
output by exactly 0.0, while perturbing hnorm_b / cnorm_b changes it
exactly as sigmoid(bias) predicts (hnorm_w has no effect, as derived).

The optimal memory-regime kernel therefore reads the 5 bias scalars,
applies sigmoid on-device (ScalarE LUT), and broadcasts to the [N, 5]
output.  Work is sharded row-wise across the 8 NeuronCores: core k
produces output rows [k*12500, (k+1)*12500).

Device kernel per core (single-engine program on ScalarE, which can
issue HWDGE DMAs — no cross-engine semaphore hops):
  1. DMA:   head_bias [128, 5] f32  HBM -> SBUF  (completion ~2 us)
  2. dummy sigmoid on scratch: hoists the sigmoid ACT-table load into
     the DMA-completion window instead of the critical path
  3. ACT:   sigmoid with a stride-0 broadcast access pattern,
            [128, 5] -> [128, 98, 5]  (12544 rows worth per core)
  4. DMA:   SBUF [128, 490] -> HBM out [128, 490], wait for receipt
Host reshapes [128, 490] -> [12544, 5], takes the first 12500 rows per
core and concatenates the 8 shards -> [100000, 5].

Measured (neuron-profile, core 0): ~14.6 us whole-NEFF exec, of which
~9.5 us is the fixed bass prologue/epilogue (engine preambles, semaphore
init, const pool, barriers) and ~5 us is the two serial DMA completion
latencies + ACT work.  Output matches the reference bit-exactly on the
real inputs and to ~1e-6 under perturbed head biases.
"""

import os
import sys

import numpy as np

# Hardcoded problem shape (kernel.py must be self-contained).
N = 100000
N_CORES = 8
ROWS_PER_CORE = N // N_CORES          # 12500
PART = 128                            # SBUF partitions
GROUPS = 98                           # 128 * 98 = 12544 >= 12500
ROWS_PAD = PART * GROUPS              # 12544

for _p in ("/opt/trn_rl_repo", "/root/.axon_site/_ro/trn_rl_repo"):
    if os.path.isdir(_p) and _p not in sys.path:
        sys.path.append(_p)

from concourse import bass, mybir  # noqa: E402
from concourse.bass import AP  # noqa: E402
from concourse.bass_utils import run_bass_kernel_spmd  # noqa: E402

# Stash of the last run's BassKernelResults (exec_time_ns etc.) so a
# harness/test can read profiling info without changing kernel()'s API.
LAST_RESULT = None


def _build_bass():
    """Per-core program: out[p, g, :] = sigmoid(head_bias[p, :])."""
    nc = bass.Bass()
    bias_in = nc.declare_dram_parameter(
        "head_bias", [PART, 5], mybir.dt.float32, isOutput=False
    )
    out_ext = nc.declare_dram_parameter(
        "out", [PART, GROUPS * 5], mybir.dt.float32, isOutput=True
    )

    with (
        nc.sbuf_tensor("sb_bias", [PART, 5], mybir.dt.float32) as sb_bias,
        nc.sbuf_tensor("sb_out", [PART, GROUPS * 5], mybir.dt.float32) as sb_out,
        nc.sbuf_tensor("sb_scratch", [PART, 1], mybir.dt.float32) as sb_scratch,
        nc.Block(no_gpsimd_drain=True) as block,
        nc.semaphore("dma_sem") as dma_sem,
    ):
        # Single-engine program on ScalarE (which can issue HWDGE DMAs):
        # no cross-engine semaphore hops at all.
        @block.scalar
        def _(scalar):
            # 1. Input DMA first, so its ~2 us completion latency overlaps
            #    with the sigmoid ACT-table load triggered by the dummy
            #    activation below (preload_activation_table trick).
            scalar.dma_start(out=sb_bias[:], in_=bias_in[:]).then_inc(dma_sem, 16)
            # Dummy sigmoid on zeroed scratch: its only purpose is to pull
            # the ACT-table load into the DMA-completion window.
            scalar.memzero(sb_scratch[:])
            scalar.drain()
            scalar.activation(
                out=sb_scratch[:],
                in_=sb_scratch[:],
                func=mybir.ActivationFunctionType.Sigmoid,
            )
            # 2. Real compute: read [128, 5] with a stride-0 middle axis,
            #    write sigmoid into [128, 98, 5] (12544 output rows).
            scalar.wait_ge(dma_sem, 16)
            in_bcast = AP(sb_bias[:].tensor, 0, [[5, PART], [0, GROUPS], [1, 5]])
            out_view = AP(
                sb_out[:].tensor, 0, [[GROUPS * 5, PART], [5, GROUPS], [1, 5]]
            )
            scalar.activation(
                out=out_view,
                in_=in_bcast,
                func=mybir.ActivationFunctionType.Sigmoid,
            )
            # 3. Output DMA; drain first so the ACT write to sb_out is
            #    retired before the DMA engines read it, then wait for the
            #    write receipt before retiring.
            scalar.drain()
            scalar.dma_start(out=out_ext[:], in_=sb_out[:]).then_inc(dma_sem, 16)
            scalar.wait_ge(dma_sem, 32)

    return nc


def kernel(**inputs) -> np.ndarray:
    global LAST_RESULT

    hnorm_b = np.asarray(inputs["hnorm_b"], dtype=np.float32).reshape(4)
    cnorm_b = np.asarray(inputs["cnorm_b"], dtype=np.float32).reshape(1)
    bias_row = np.concatenate([cnorm_b, hnorm_b])  # [5]: comp, rmav0..3
    head_bias = np.ascontiguousarray(
        np.broadcast_to(bias_row[None, :], (PART, 5)), dtype=np.float32
    )

    nc = _build_bass()
    # Shard rows across the 8 cores; the row->value map is constant in n,
    # so every core receives the same (replicated) bias tile and computes
    # its 12544-row slab; the host keeps 12500 rows per core.
    in_maps = [{"head_bias": head_bias} for _ in range(N_CORES)]
    trace = os.environ.get("KERNEL_TRACE", "0") == "1"
    res = run_bass_kernel_spmd(
        nc, in_maps, core_ids=list(range(N_CORES)), trace=trace
    )
    LAST_RESULT = res

    shards = []
    for k in range(N_CORES):
        tile = np.asarray(res.results[k]["out"], dtype=np.float32)
        shards.append(tile.reshape(ROWS_PAD, 5)[:ROWS_PER_CORE])
    return np.ascontiguousarray(np.concatenate(shards, axis=0))


if __name__ == "__main__":
    demo = {
        "hnorm_b": np.zeros((4, 1), np.float32),
        "cnorm_b": np.zeros((1,), np.float32),
    }
    out = kernel(**demo)
    print("out", out.shape, out.dtype, "max|out-0.5| =", np.abs(out - 0.5).max())
